# revision 22
# baseline (speedup 1.0000x reference)
"""Trainium2 Bass kernel for the PCNN (piecewise-CNN) bag-classification model.

Design (v2):
  - Balanced bag-boundary sharding over 8 cores (no collective; host concat).
  - Per-core vocabulary compaction (unique tokens < 32768) so the fast
    dma_gather (int16 indices) can be used.
  - fp8 dma_gather(transpose=True) delivers the word embeddings directly in
    pair-interleaved channel-major layout (no PE transposes at all).
  - conv1d(k=3) as 4 DoubleRow fp8 matmuls per (subgroup, filter-chunk):
    3 full taps over channels 0..255 + one packed pass carrying the
    channel-256..299 leftovers (pre-shifted per tap), the positional
    embeddings and the piece-0 mask channel.
  - Inverted masks: scored_j = conv + 48*(m_j - 1) so in-piece values keep
    full precision in bf16; out-of-piece values sit at ~-48.
  - Piecewise max: scalar copies PSUM->SBUF bf16 (phase 0), DVE reduces at
    4x + one bf16 delta add, GpSimd builds phase-2 array from PSUM.
  - Dense + bag-mean + softmax per core on its own bags.
"""

import os
import sys

for _p in ("/opt/trn_rl_repo",):
    if _p not in sys.path:
        sys.path.insert(0, _p)

import numpy as np
import ml_dtypes

# ---------------- problem constants ----------------
N = 2048
L = 120
LP = 122
NCORES = 8
NF = 230
NREL = 53
NBAGS = 256
VOCAB = 100000
WD = 300
PD = 5

SGW = 488            # tokens per subgroup (4 sentences x 122)
NIDX = 3968          # gather idxs per 8-sg group (488*8=3904 padded to %128)
TBL = 32768          # compacted table rows
ES = 512             # table row, fp8 elements (300 used)
MB = 4.0             # inverted-mask magnitude (exact in fp8/fp16; small
                     # so fp16 phase arrays keep mantissa after +/-MB round-trip)
FCH = [(0, 128, 128), (128, 102, 112)]  # (f0, fw_real, fw_pad)

FP8 = ml_dtypes.float8_e4m3
BF16 = ml_dtypes.bfloat16

_PROGRAM_CACHE = {}
LAST_RESULT = None


def _build_program(nsg, ngrp, bags_cap, nchunk):
    import concourse.bass as bass
    import concourse.mybir as mybir
    import concourse.tile as tile
    from concourse import bacc

    f32 = mybir.dt.float32
    bf16 = mybir.dt.bfloat16
    fp8 = mybir.dt.float8e4
    i16 = mybir.dt.int16
    f16 = mybir.dt.float16
    AF = mybir.ActivationFunctionType
    AX = mybir.AxisListType
    ALU = mybir.AluOpType
    DR = mybir.MatmulPerfMode.DoubleRow

    ns_pad = 4 * nsg

    nc = bacc.Bacc(
        "TRN2", target_bir_lowering=False, debug=False, num_devices=NCORES,
        num_swdge_queues=4,
    )

    wembc_d = nc.dram_tensor("wembc", [TBL, ES], fp8, kind="ExternalInput").ap()
    idx_d = nc.dram_tensor("idxw", [ngrp, 128, NIDX // 16], i16,
                           kind="ExternalInput").ap()
    ppf_d = nc.dram_tensor("ppf", [ngrp, 16, 2 * NIDX], fp8,
                           kind="ExternalInput").ap()
    maskd_d = nc.dram_tensor("maskd", [ngrp, 1, 2 * 8 * 480], f16,
                             kind="ExternalInput").ap()
    wdr_d = nc.dram_tensor("wdr", [3, 128, 2 * 240], fp8,
                           kind="ExternalInput").ap()
    wp_d = nc.dram_tensor("wp", [112, 2 * 240], fp8, kind="ExternalInput").ap()
    dwt_d = nc.dram_tensor("dwt", [128, 6 * NREL], f16,
                           kind="ExternalInput").ap()
    actb_d = nc.dram_tensor("actb", [128, 2], f32, kind="ExternalInput").ap()
    dbias_d = nc.dram_tensor("dbias", [1, NREL], f16, kind="ExternalInput").ap()
    snorm_d = nc.dram_tensor("snorm", [nchunk * 128, bags_cap], f16,
                             kind="ExternalInput").ap()
    out_d = nc.dram_tensor("out", [bags_cap, NREL], f32, kind="ExternalOutput").ap()
    debug = bool(int(os.environ.get("KERNEL_DEBUG", "0")))
    stage = int(os.environ.get("KERNEL_STAGE", "4"))
    if debug:
        dbg_ps = nc.dram_tensor("dbg_ps", [128, 486], f32, kind="ExternalOutput").ap()
        dbg_cp = nc.dram_tensor("dbg_cp", [128, 480], f16, kind="ExternalOutput").ap()
        dbg_pool = nc.dram_tensor("dbg_pool", [2, 128, 3, ns_pad], f16,
                                  kind="ExternalOutput").ap()

    with tile.TileContext(nc) as tc:
        import contextlib

        ctx = contextlib.ExitStack()
        with ctx:
            singles = ctx.enter_context(tc.tile_pool(name="singles", bufs=1))

            wdr_sb = [singles.tile([128, 2, 240], fp8, name=f"wdr{t}")
                      for t in range(3)]
            wp_sb = singles.tile([112, 2, 240], fp8)
            idx_sb = singles.tile([128, ngrp, NIDX // 16], i16)
            dwt_sb = singles.tile([128, 6 * NREL], f16)
            actb_sb = singles.tile([128, 2], f32)
            dbias_sb = singles.tile([1, NREL], f16)
            snorm_sb = [singles.tile([128, bags_cap], f16, name=f"sn{c}")
                        for c in range(nchunk)]
            ones_sb = singles.tile([1, 128], f16)
            pooled = [singles.tile([128, 3, ns_pad], f16, name=f"pool{c}")
                      for c in range(2)]

            for t in range(3):
                nc.sync.dma_start(out=wdr_sb[t][:, :, :], in_=wdr_d[t, :, :])
            nc.sync.dma_start(out=wp_sb[:, :, :], in_=wp_d[:, :])
            for g in range(ngrp):
                nc.sync.dma_start(out=idx_sb[:, g, :], in_=idx_d[g, :, :])
            nc.sync.dma_start(out=dwt_sb[:, :], in_=dwt_d[:, :])
            nc.sync.dma_start(out=actb_sb[:, :], in_=actb_d[:, :])
            nc.sync.dma_start(out=dbias_sb[:, :], in_=dbias_d[:, :])
            for c in range(nchunk):
                nc.sync.dma_start(out=snorm_sb[c][:, :],
                                  in_=snorm_d[c * 128:(c + 1) * 128, :])
            nc.vector.memset(ones_sb[:, :], 1.0)

            gt_pool = ctx.enter_context(tc.tile_pool(name="gt", bufs=4))
            p_pool = ctx.enter_context(tc.tile_pool(name="pp", bufs=2))
            mk_pool = ctx.enter_context(tc.tile_pool(name="mk", bufs=2))
            cp_pool = ctx.enter_context(tc.tile_pool(name="cp", bufs=8))
            cv_psum = ctx.enter_context(tc.tile_pool(name="cv", bufs=6, space="PSUM"))

            for g in range(ngrp):
                sg0 = 8 * g
                sgn = min(8, nsg - sg0)

                gt = gt_pool.tile([128, 4, NIDX], fp8, tag="gt")
                nc.gpsimd.dma_gather(
                    out_ap=gt[:, :, :],
                    in_ap=wembc_d[:, :],
                    idxs_ap=idx_sb[:, g, :],
                    num_idxs=NIDX,
                    num_idxs_reg=NIDX,
                    elem_size=ES,
                    transpose=True,
                    single_packet=False,
                    queue_num=0,
                )
                # packed tile: leftover word chans (pre-shifted per tap) +
                # pf/mask rows from DRAM
                pt = p_pool.tile([112, 2 * NIDX], fp8, tag="pp")
                nc.sync.dma_start(out=pt[96:112, :], in_=ppf_d[g, :, :])
                g1v = gt[:, 2:4, :].bitcast(i16)    # [128, 2, NIDX//2] int16
                ptv = pt[:, :].bitcast(i16)         # [112, NIDX] int16
                for t in range(3):
                    # copy 32 rows: rows 22..31 are table-pad zeros, filling
                    # the hole so the packed matmul never reads stale (NaN)
                    # fp8 bytes
                    src = bass.AP(
                        tensor=g1v.tensor, offset=g1v.offset + t,
                        ap=[[g1v.ap[0][0], 32], [1, NIDX - 2]],
                    )
                    nc.scalar.copy(
                        out=ptv[32 * t:32 * t + 32, 0:NIDX - 2], in_=src,
                    )

                mk = mk_pool.tile([128, 2, 8 * 480], f16, tag="mk")
                mkb = mk[:, 0, 0]
                mk_flat = bass.AP(
                    tensor=mkb.tensor, offset=mkb.offset,
                    ap=[mkb.ap[0], [1, 2 * 8 * 480]],
                )
                nc.sync.dma_start(
                    out=mk_flat,
                    in_=maskd_d[g, :, :].to_broadcast((128, 2 * 8 * 480)),
                )

                for sgl in range(sgn if stage >= 2 else 0):
                    for fc, (f0, fw, fwp) in enumerate(FCH):
                        ps = cv_psum.tile([128, 486], f32, tag="cv",
                                          name=f"cv{g}_{sgl}_{fc}")
                        gtb = gt[:, 0, 0]
                        for t in range(3):
                            rhs = bass.AP(
                                tensor=gtb.tensor,
                                offset=gtb.offset + 2 * (SGW * sgl + t),
                                ap=[gtb.ap[0], [1, 2], [2, 486]],
                            )
                            nc.tensor.matmul(
                                out=ps[0:fwp, :],
                                lhsT=wdr_sb[t][:, :, f0:f0 + fwp],
                                rhs=rhs,
                                start=(t == 0), stop=False,
                                perf_mode=DR, skip_group_check=True,
                            )
                        pb = pt[0:112, 0]
                        rhsp = bass.AP(
                            tensor=pb.tensor,
                            offset=pb.offset + 2 * (SGW * sgl),
                            ap=[[pb.ap[0][0], 112], [1, 2], [2, 486]],
                        )
                        nc.tensor.matmul(
                            out=ps[0:fwp, :],
                            lhsT=wp_sb[0:112, :, f0:f0 + fwp],
                            rhs=rhsp,
                            start=False, stop=True,
                            perf_mode=DR, skip_group_check=True,
                        )

                        sg = sg0 + sgl
                        s0 = 4 * sg
                        if stage < 3:
                            # drain PSUM so the accumulation group completes
                            drain = cp_pool.tile([128, 4, 120], f16, tag="cp",
                                                 name=f"dr{g}_{sgl}_{fc}")
                            nc.vector.tensor_copy(out=drain[0:fw, :, :],
                                                  in_=ps[0:fw, 0:480])
                            continue
                        # strided PSUM view: 122-token stride -> dense 480
                        pb0 = ps[0:fw, 0]
                        psv = bass.AP(
                            tensor=pb0.tensor, offset=pb0.offset,
                            ap=[pb0.ap[0], [122, 4], [1, 120]],
                        )
                        cp0 = cp_pool.tile([128, 4, 120], f16, tag="cp",
                                           name=f"cp0_{g}_{sgl}_{fc}")
                        nc.scalar.copy(out=cp0[0:fw, :, :], in_=psv)
                        # phase-2 array from the phase-0 copy (gpsimd cannot
                        # access PSUM on trn2); split gpsimd/DVE 5:3
                        cp2 = cp_pool.tile([128, 4, 120], f16, tag="cp",
                                           name=f"cp2_{g}_{sgl}_{fc}")
                        nc.vector.tensor_tensor(
                            out=cp2[0:fw, :, :],
                            in0=cp0[0:fw, :, :],
                            in1=mk[0:fw, 1, sgl * 480:(sgl + 1) * 480],
                            op=ALU.add,
                        )
                        if debug and g == 0 and sgl == 0 and fc == 0:
                            dtile = singles.tile([128, 486], f32, name="dbgt")
                            nc.vector.tensor_copy(out=dtile[:, :], in_=ps[:, :])
                            nc.sync.dma_start(out=dbg_ps[:, :], in_=dtile[:, :])
                            nc.sync.dma_start(out=dbg_cp[:, :],
                                              in_=cp0[:, :, :])
                        nc.vector.reduce_max(
                            out=pooled[fc][0:fw, 0, s0:s0 + 4],
                            in_=cp0[0:fw, :, :], axis=AX.X,
                        )
                        nc.vector.tensor_tensor(
                            out=cp0[0:fw, :, :], in0=cp0[0:fw, :, :],
                            in1=mk[0:fw, 0, sgl * 480:(sgl + 1) * 480],
                            op=ALU.add,
                        )
                        nc.vector.reduce_max(
                            out=pooled[fc][0:fw, 1, s0:s0 + 4],
                            in_=cp0[0:fw, :, :], axis=AX.X,
                        )
                        nc.vector.reduce_max(
                            out=pooled[fc][0:fw, 2, s0:s0 + 4],
                            in_=cp2[0:fw, :, :], axis=AX.X,
                        )

            # ---------------- tail ----------------
            if stage < 4:
                res0 = singles.tile([128, NREL], f32, name="res0")
                nc.vector.memset(res0[:, :], 0.0)
                nc.sync.dma_start(out=out_d[:, :], in_=res0[0:bags_cap, :])
            if debug:
                for fc in range(2):
                    nc.sync.dma_start(out=dbg_pool[fc, :, :, :],
                                      in_=pooled[fc][:, :, :])
            pr = [singles.tile([128, 3, ns_pad], f16, name=f"pr{c}")
                  for c in range(2)]
            for fc in range(2 if stage >= 4 else 0):
                nc.scalar.activation(
                    out=pr[fc][:, :, :], in_=pooled[fc][:, :, :],
                    func=AF.Relu, bias=actb_sb[:, fc:fc + 1], scale=1.0,
                )

            lgs = [singles.tile([128, NREL], f16, name=f"lgs{c}")
                   for c in range(nchunk)]
            for c in range(nchunk if stage >= 4 else 0):
                cs = min(128, ns_pad - 128 * c)
                lg_ps = cv_psum.tile([128, NREL], f32, tag="cv", name=f"lg{c}")
                nmm = 0
                for j in range(3):
                    for fc, (f0, fw, fwp) in enumerate(FCH):
                        nc.tensor.matmul(
                            out=lg_ps[0:cs, :],
                            lhsT=pr[fc][0:fw, j, 128 * c:128 * c + cs],
                            rhs=dwt_sb[0:fw, (j * 2 + fc) * NREL:
                                       (j * 2 + fc + 1) * NREL],
                            start=(nmm == 0), stop=False,
                            skip_group_check=True,
                        )
                        nmm += 1
                nc.tensor.matmul(
                    out=lg_ps[0:cs, :],
                    lhsT=ones_sb[0:1, 0:cs],
                    rhs=dbias_sb[0:1, :],
                    start=False, stop=True, skip_group_check=True,
                )
                nc.scalar.copy(out=lgs[c][0:cs, :], in_=lg_ps[0:cs, :])

            bg_ps = cv_psum.tile([128, NREL], f32, tag="cv", name="bg")
            for c in range(nchunk if stage >= 4 else 0):
                cs = min(128, ns_pad - 128 * c)
                nc.tensor.matmul(
                    out=bg_ps[0:bags_cap, :],
                    lhsT=snorm_sb[c][0:cs, :],
                    rhs=lgs[c][0:cs, :],
                    start=(c == 0), stop=(c == nchunk - 1),
                    skip_group_check=True,
                )

            if stage >= 4:
                t = singles.tile([128, NREL], f32, name="sm")
                nc.vector.tensor_copy(out=t[0:bags_cap, :],
                                      in_=bg_ps[0:bags_cap, :])
                nmax = singles.tile([128, 1], f32, name="nmax")
                nc.vector.reduce_max(out=nmax[0:bags_cap, :],
                                     in_=t[0:bags_cap, :],
                                     axis=AX.X, negate=True)
                ex = singles.tile([128, NREL], f32, name="ex")
                nc.scalar.activation(out=ex[0:bags_cap, :], in_=t[0:bags_cap, :],
                                     func=AF.Exp, bias=nmax[0:bags_cap, :],
                                     scale=1.0)
                ssum = singles.tile([128, 1], f32, name="ssum")
                nc.vector.reduce_sum(out=ssum[0:bags_cap, :],
                                     in_=ex[0:bags_cap, :], axis=AX.X)
                rcp = singles.tile([128, 1], f32, name="rcp")
                nc.vector.reciprocal(out=rcp[0:bags_cap, :],
                                     in_=ssum[0:bags_cap, :])
                res = singles.tile([128, NREL], f32, name="res")
                nc.vector.tensor_scalar_mul(res[0:bags_cap, :],
                                            ex[0:bags_cap, :],
                                            rcp[0:bags_cap, :])
                nc.sync.dma_start(out=out_d[:, :], in_=res[0:bags_cap, :])

    nc.compile()
    return nc


def _pad_edge(a):
    return np.concatenate([a[:, :1], a, a[:, -1:]], axis=1)


def kernel(**inputs):
    global LAST_RESULT
    sentences = np.asarray(inputs["sentences"]).astype(np.int64)
    pos1 = np.asarray(inputs["pos1"]).astype(np.int64)
    pos2 = np.asarray(inputs["pos2"]).astype(np.int64)
    masks = np.asarray(inputs["masks"]).astype(np.float32)
    bag_ids = np.asarray(inputs["bag_ids"]).astype(np.int64)
    word_emb = np.asarray(inputs["word_emb"]).astype(np.float32)
    pf1_emb = np.asarray(inputs["pf1_emb"]).astype(np.float32)
    pf2_emb = np.asarray(inputs["pf2_emb"]).astype(np.float32)
    conv_w = np.asarray(inputs["conv_w"]).astype(np.float32)
    conv_b = np.asarray(inputs["conv_b"]).astype(np.float32)
    dense_w = np.asarray(inputs["dense_w"]).astype(np.float32)
    dense_b = np.asarray(inputs["dense_b"]).astype(np.float32)

    # ---- balanced bag-boundary sharding ----
    counts = np.bincount(bag_ids, minlength=NBAGS)
    cum = np.concatenate([[0], np.cumsum(counts)])  # [257]
    B = [0]
    for r in range(1, NCORES):
        B.append(int(np.argmin(np.abs(cum - N * r // NCORES))))
    B.append(NBAGS)
    for r in range(1, NCORES + 1):
        B[r] = max(B[r], B[r - 1])
    S = [int(cum[b]) for b in B]
    cnt = [S[r + 1] - S[r] for r in range(NCORES)]
    ncap = max(max(cnt), 1)
    nsg = (ncap + 3) // 4
    ns_pad = 4 * nsg
    ngrp = (nsg + 7) // 8
    bags_cap = max(B[r + 1] - B[r] for r in range(NCORES))
    nchunk = (ns_pad + 127) // 128

    key = (nsg, ngrp, bags_cap, nchunk,
           os.environ.get("KERNEL_STAGE", "4"))
    if key not in _PROGRAM_CACHE:
        _PROGRAM_CACHE[key] = _build_program(nsg, ngrp, bags_cap, nchunk)
    nc = _PROGRAM_CACHE[key]

    # ---- shared parameter prep ----
    wemb8 = word_emb.astype(FP8)  # [VOCAB, 300]

    # conv weights: full-tap DoubleRow [3, 128, 2, 240]
    wdr = np.zeros((3, 128, 2, 240), np.float32)
    for t in range(3):
        for i in range(2):
            wdr[t, :, i, :NF] = conv_w[:, i:256:2, t].T
    wdr = wdr.astype(FP8)

    # packed pass weights [112, 2, 240]; tap t word-leftovers at rows 32t,
    # pf at 96+5t, mask at 111 (engine partition bases must be 32-aligned)
    wp = np.zeros((112, 2, 240), np.float32)
    for t in range(3):
        for i in range(2):
            wp[32 * t:32 * t + 22, i, :NF] = conv_w[:, 256 + i:300:2, t].T
            wp[96 + 5 * t:96 + 5 * t + 5, i, :NF] = conv_w[:, 300 + i:310:2, t].T
    wp[111, 0, :NF] = 1.0  # mask channel (center tap rides in packed pass)
    wp = wp.astype(FP8)

    dwt = np.zeros((128, 6 * NREL), np.float32)
    for j in range(3):
        for fc, (f0, fw, fwp) in enumerate(FCH):
            dwt[:fw, (j * 2 + fc) * NREL:(j * 2 + fc + 1) * NREL] = \
                dense_w[:, j * NF + f0:j * NF + f0 + fw].T
    dwt = dwt.astype(np.float16)

    actb = np.zeros((128, 2), np.float32)
    for fc, (f0, fw, fwp) in enumerate(FCH):
        actb[:fw, fc] = conv_b[f0:f0 + fw]

    dbias = dense_b.reshape(1, NREL).astype(np.float16)
    fcounts = np.maximum(counts.astype(np.float32), 1.0)

    in_maps = []
    for r in range(NCORES):
        s0r, s1r = S[r], S[r + 1]
        nreal = s1r - s0r
        sent = np.zeros((ns_pad, L), np.int64)
        sent[:nreal] = sentences[s0r:s1r]
        p1 = np.zeros((ns_pad, L), np.int64)
        p1[:nreal] = pos1[s0r:s1r]
        p2 = np.zeros((ns_pad, L), np.int64)
        p2[:nreal] = pos2[s0r:s1r]
        mcore = np.zeros((ns_pad, 3, L), np.float32)
        mcore[:nreal] = masks[s0r:s1r]

        sp = _pad_edge(sent)          # [ns_pad, 122]
        uniq, inv = np.unique(sp, return_inverse=True)
        assert len(uniq) <= TBL, f"unique tokens {len(uniq)} > {TBL}"
        inv = inv.reshape(sp.shape).astype(np.int16)

        wembc = np.zeros((TBL, ES), FP8)
        wembc[:len(uniq), :WD] = wemb8[uniq]

        stream = inv.reshape(nsg, SGW)  # [nsg, 488]

        idxw = np.zeros((ngrp, 128, NIDX // 16), np.int16)
        for g in range(ngrp):
            sgn = min(8, nsg - 8 * g)
            toks = np.zeros(NIDX, np.int16)
            toks[:sgn * SGW] = stream[8 * g:8 * g + sgn].ravel()
            w16 = toks.reshape(NIDX // 16, 16).T  # [16, 248]
            idxw[g] = np.tile(w16, (8, 1))

        # pf slot values and mask0 slot values
        p1p = _pad_edge(p1)
        p2p = _pad_edge(p2)
        pfv = np.concatenate([pf1_emb[p1p], pf2_emb[p2p]], axis=2)
        pfflat = pfv.reshape(nsg, SGW, 10)
        m0 = np.zeros((ns_pad, LP), np.float32)
        m0[:, 1:L + 1] = mcore[:, 0, :]
        m0flat = m0.reshape(nsg, SGW)

        ppf = np.zeros((ngrp, 16, 2 * NIDX), np.float32)
        for g in range(ngrp):
            sgn = min(8, nsg - 8 * g)
            nrt = sgn * SGW
            src = np.zeros((NIDX + 2, 10), np.float32)
            src[:nrt] = pfflat[8 * g:8 * g + sgn].reshape(-1, 10)
            msrc = np.zeros(NIDX + 2, np.float32)
            msrc[:nrt] = m0flat[8 * g:8 * g + sgn].ravel()
            for t in range(3):
                for p in range(5):
                    ppf[g, 5 * t + p, 0::2] = src[t:t + NIDX, 2 * p]
                    ppf[g, 5 * t + p, 1::2] = src[t:t + NIDX, 2 * p + 1]
            ppf[g, 15, 0::2] = MB * (msrc[1:1 + NIDX] - 1.0)
        ppf = ppf.astype(FP8)

        # mask deltas, dense-480 layout [ngrp, 2*8*480]
        md = np.zeros((ngrp, 2, 8 * 480), np.float32)
        d1 = (MB * (mcore[:, 1, :] - mcore[:, 0, :])).reshape(nsg, 4 * L)
        d2 = (MB * (mcore[:, 2, :] - mcore[:, 0, :])).reshape(nsg, 4 * L)
        for g in range(ngrp):
            sgn = min(8, nsg - 8 * g)
            md[g, 0, :sgn * 480] = d1[8 * g:8 * g + sgn].ravel()
            md[g, 1, :sgn * 480] = d2[8 * g:8 * g + sgn].ravel()
        md = md.reshape(ngrp, 1, 2 * 8 * 480).astype(np.float16)

        snorm = np.zeros((nchunk * 128, bags_cap), np.float32)
        bags = bag_ids[s0r:s1r]
        snorm[np.arange(nreal), bags - B[r]] = 1.0 / fcounts[bags]
        snorm = snorm.astype(np.float16)

        in_maps.append({
            "wembc": wembc,
            "idxw": idxw,
            "ppf": ppf,
            "maskd": md,
            "wdr": wdr.reshape(3, 128, 2 * 240),
            "wp": wp.reshape(112, 2 * 240),
            "dwt": dwt,
            "actb": actb,
            "dbias": dbias,
            "snorm": snorm,
        })

    from concourse.bass_utils import run_bass_kernel_spmd

    trace = bool(int(os.environ.get("KERNEL_TRACE", "0")))
    res = run_bass_kernel_spmd(
        nc, in_maps, core_ids=list(range(NCORES)), trace=trace
    )
    LAST_RESULT = res

    out = np.zeros((NBAGS, NREL), np.float32)
    for r in range(NCORES):
        nb = B[r + 1] - B[r]
        if nb > 0:
            out[B[r]:B[r + 1]] = res.results[r]["out"][:nb].astype(np.float32)
    return out


if __name__ == "__main__":
    d = np.load("/root/problem/ref_inputs.npz")
    out = kernel(**{k: d[k] for k in d.files})
    print("out", out.shape, out.dtype)
